# revision 21
# baseline (speedup 1.0000x reference)
"""GQA attention (RoPE, causal) on 8 TRN2 NeuronCores.

Sharding: core c = (b, g) with b = c // 4 (batch), g = c % 4 (kv-group).
Each core computes 4 query heads + 1 kv head of attention for one batch
element, plus its slice of the output projection; the host sums the 4
partial outputs per batch (row-parallel wo unshard).

Layout tricks:
- wq/wk columns are permuted on the host (per-head de-interleave of RoPE
  pairs). Scores are invariant to a shared per-head permutation of
  head_dim, and this makes RoPE contiguous-block elementwise ops.
- Scores are computed transposed, ST[k, t] = K_rot @ Q_rot^T, so the AV
  matmul consumes exp(ST) directly with V in natural [t, d] layout and a
  ones-column in V yields the softmax denominators for free.
- Matmuls run in bf16 (f32 accumulate); x and the weights are cast to
  bf16 on the host. Softmax skips the max-subtraction (scores are small,
  exp is safe in f32) and normalization is batched per head.
"""
import sys

sys.path.insert(0, "/opt/trn_rl_repo")
import ml_dtypes
import numpy as np

import concourse.bass as bass  # noqa: F401
import concourse.tile as tile
from concourse import bacc, mybir
from concourse.bass_utils import run_bass_kernel_spmd

F32 = mybir.dt.float32
BF16 = mybir.dt.bfloat16

B, T, DIM = 2, 2048, 1024
H, KV, HD = 16, 4, 64
NQ = H // KV          # q heads per core
THETA = 10000.0
SCALE = HD ** -0.5
NCORES = 8
QCH = 512             # q-chunk (free dim of scores/AV matmuls)
NQC = T // QCH        # 4 q-chunks
NKT = T // 128        # 16 k-tiles


def build_nc():
    nc = bacc.Bacc(None, target_bir_lowering=False)
    x_d = nc.declare_dram_parameter("x", [T, DIM], BF16, isOutput=False)
    wqkv_d = nc.declare_dram_parameter("wqkv", [DIM, 384], BF16, isOutput=False)
    wo_d = nc.declare_dram_parameter("wo", [256, DIM], BF16, isOutput=False)
    cosq_d = nc.declare_dram_parameter("cosq", [128, T], F32, isOutput=False)
    sinq_d = nc.declare_dram_parameter("sinq", [128, T], F32, isOutput=False)
    triu_d = nc.declare_dram_parameter("triu", [128, 128], BF16, isOutput=False)
    id_d = nc.declare_dram_parameter("ident", [128, 128], BF16, isOutput=False)
    out_d = nc.declare_dram_parameter("out", [T, DIM], BF16, isOutput=True)

    XCH = 512
    NCH = T // XCH        # 4 chunks
    NKPC = 4              # k-tiles per chunk

    with tile.TileContext(nc) as tc:
        with (
            tc.tile_pool(name="persist", bufs=1) as pp,
            tc.tile_pool(name="vpool", bufs=16) as vp,
            tc.tile_pool(name="xtp", bufs=2) as pxt,
            tc.tile_pool(name="chunk", bufs=2) as pch,
            tc.tile_pool(name="attn", bufs=4) as pb,
            tc.tile_pool(name="small", bufs=2) as pb2,
            tc.tile_pool(name="norm", bufs=1) as pb1,
            tc.tile_pool(name="pst", bufs=2, space="PSUM") as pst,
            tc.tile_pool(name="pav", bufs=2, space="PSUM") as pav,
            tc.tile_pool(name="pproj", bufs=2, space="PSUM") as ppj,
        ):
            # ---- persistent tiles ----
            wqkv_s = pp.tile([128, 8, 384], BF16, tag="wqkv_s")
            nc.sync.dma_start(wqkv_s[:], wqkv_d.rearrange("(k p) c -> p k c", p=128))

            # per-chunk rotated q/k, unnormalized out, normalized out
            qrot = [
                [
                    pp.tile([128, XCH], BF16, tag=f"qr{hp}_{c}", name=f"qr{hp}_{c}")
                    for c in range(NCH)
                ]
                for hp in range(2)
            ]
            krotc = [
                pp.tile([128, XCH], BF16, tag=f"kr{c}", name=f"kr{c}")
                for c in range(NCH)
            ]
            outU = [
                [
                    pp.tile([128, XCH], F32, tag=f"oU{hp}_{c}", name=f"oU{hp}_{c}")
                    for c in range(NCH)
                ]
                for hp in range(2)
            ]
            outTn = [
                [
                    pp.tile([128, XCH], BF16, tag=f"oT{hp}_{c}", name=f"oT{hp}_{c}")
                    for c in range(NCH)
                ]
                for hp in range(2)
            ]
            v_tiles = [
                vp.tile([128, HD + 1], BF16, tag="v", name=f"v{i}")
                for i in range(NKT)
            ]
            ones128 = pp.tile([128, 1], BF16, tag="ones128")
            nc.vector.memset(ones128[:], 1.0)
            ones_row = pp.tile([128, HD], BF16, tag="ones_row")
            nc.vector.memset(ones_row[:], 1.0)
            den_all = pb1.tile([97, T], F32, tag="den_all")
            invh = [
                pb1.tile([1, T], F32, tag=f"invh{h}", name=f"invh{h}")
                for h in range(NQ)
            ]

            id_s = pp.tile([128, 128], BF16, tag="ident")
            nc.sync.dma_start(id_s[:], id_d[:])

            # x loaded via regular DMAs; transposed on PE per chunk
            xins = []
            for nch in range(NCH):
                xin = [
                    pxt.tile([128, DIM], BF16, tag="xin", name=f"xin{i}", bufs=6)
                    for i in range(4)
                ]
                for i in range(4):
                    r0 = nch * XCH + i * 128
                    nc.sync.dma_start(xin[i][:], x_d[r0 : r0 + 128, :])
                xins.append(xin)

            cosq = pp.tile([128, T], F32, tag="cosq")
            sinq = pp.tile([128, T], F32, tag="sinq")
            triu = pp.tile([128, 128], BF16, tag="triu")
            nc.sync.dma_start(cosq[:], cosq_d[:])
            nc.sync.dma_start(sinq[:], sinq_d[:])
            nc.sync.dma_start(triu[:], triu_d[:])
            wo_s = pp.tile([128, 2, DIM], BF16, tag="wo_s")
            nc.sync.dma_start(wo_s[:], wo_d.rearrange("(k p) c -> p k c", p=128))

            # ---- phase A: per-chunk projection + rope + v build ----
            for nch in range(NCH):
                cs = slice(nch * XCH, (nch + 1) * XCH)
                ccs = cosq[:, cs]
                scs = sinq[:, cs]
                xtc = [
                    pxt.tile([128, XCH], BF16, tag=f"xt{d}", name=f"xt{d}", bufs=2)
                    for d in range(8)
                ]
                for d in range(8):
                    for i in range(4):
                        ptx = ppj.tile([128, 128], BF16, tag="pq", name="ptx")
                        nc.tensor.transpose(
                            ptx[:],
                            xins[nch][i][:, d * 128 : (d + 1) * 128],
                            id_s[:],
                        )
                        dst = xtc[d][:, i * 128 : (i + 1) * 128]
                        if (d * 4 + i) % 2 == 0:
                            nc.vector.tensor_copy(dst, ptx[:])
                        else:
                            nc.scalar.copy(dst, ptx[:])
                t0c = pch.tile([128, XCH], F32, tag="t0c", name="t0c")
                t1c = pch.tile([128, XCH], F32, tag="t1c", name="t1c")
                tkc = pch.tile([64, XCH], F32, tag="tkc", name="tkc")
                vTc = pch.tile([64, XCH], BF16, tag="vTc", name="vTc")
                for m in range(3):
                    pq = ppj.tile([128, XCH], F32, tag="pq", name="pq")
                    for k in range(8):
                        nc.tensor.matmul(
                            pq[:],
                            wqkv_s[:, k, m * 128 : (m + 1) * 128],
                            xtc[k][:],
                            start=(k == 0),
                            stop=(k == 7),
                        )
                    if m == 0:
                        nc.scalar.copy(t0c[:], pq[:])
                    elif m == 1:
                        nc.scalar.copy(t1c[:], pq[:])
                    else:
                        nc.vector.tensor_copy(tkc[0:64, :], pq[0:64, :])
                        nc.vector.tensor_copy(vTc[:, :], pq[64:128, :])

                # RoPE (q) for this chunk
                sA = pch.tile([128, XCH], F32, tag="sA", name="sA")
                sB = pch.tile([128, XCH], F32, tag="sB", name="sB")
                nc.vector.tensor_mul(sA[:], t0c[:], ccs)
                nc.vector.tensor_mul(sB[:], t1c[:], scs)
                nc.vector.tensor_sub(sA[:], sA[:], sB[:])      # rotated evens
                nc.vector.tensor_mul(sB[:], t0c[:], scs)
                nc.vector.tensor_mul(t0c[:], t1c[:], ccs)
                nc.vector.tensor_add(sB[:], sB[:], t0c[:])     # rotated odds
                for h in range(NQ):
                    qt = qrot[h // 2][nch]
                    roff = 64 * (h % 2)
                    src_e = sA[32 * h : 32 * h + 32, :]
                    src_o = sB[32 * h : 32 * h + 32, :]
                    if h % 2 == 0:
                        nc.vector.tensor_copy(qt[roff : roff + 32, :], src_e)
                        nc.gpsimd.tensor_copy(qt[roff + 32 : roff + 64, :], src_o)
                    else:
                        nc.scalar.copy(qt[roff : roff + 32, :], src_e)
                        nc.gpsimd.tensor_copy(qt[roff + 32 : roff + 64, :], src_o)

                # RoPE (k) for this chunk, duplicated rows 64:128
                krc = krotc[nch]
                nc.vector.tensor_mul(sA[0:32, :], tkc[0:32, :], ccs[0:32, :])
                nc.vector.tensor_mul(sB[0:32, :], tkc[32:64, :], scs[32:64, :])
                nc.vector.tensor_sub(krc[0:32, :], sA[0:32, :], sB[0:32, :])
                nc.vector.tensor_mul(sA[0:32, :], tkc[0:32, :], scs[0:32, :])
                nc.vector.tensor_mul(sB[0:32, :], tkc[32:64, :], ccs[32:64, :])
                nc.vector.tensor_add(krc[32:64, :], sA[0:32, :], sB[0:32, :])
                nc.gpsimd.tensor_copy(krc[64:128, :], krc[0:64, :])

                # V tiles for this chunk
                for ii in range(NKPC):
                    i = nch * NKPC + ii
                    pv = ppj.tile([128, HD], BF16, tag="pq", name="pv")
                    nc.tensor.transpose(
                        pv[:],
                        vTc[:, ii * 128 : (ii + 1) * 128],
                        id_s[0:64, 0:64],
                    )
                    nc.vector.tensor_copy(v_tiles[i][:, 0:HD], pv[:])
                    nc.scalar.copy(v_tiles[i][:, HD : HD + 1], ones128[:])

            # ---- phase B: attention (qc-outer, 2-stage skew, paired heads) ----
            for qc in range(NQC):
                nkt = NKPC * (qc + 1)
                for hp in range(2):
                    po = [
                        pav.tile([HD + 1, QCH], F32, tag="po", name=f"po{i}")
                        for i in range(2)
                    ]

                    def emit_avs(entry):
                        ktv, col0, et2 = entry
                        nc.tensor.matmul(
                            po[0][:, col0:QCH],
                            v_tiles[ktv][:],
                            et2[:, col0:QCH],
                            start=(ktv == 0),
                            stop=(ktv == nkt - 1),
                        )
                        nc.tensor.matmul(
                            po[1][:, col0:QCH],
                            v_tiles[ktv][:],
                            et2[:, QCH : 2 * QCH - col0],
                            start=(ktv == 0),
                            stop=(ktv == nkt - 1),
                        )

                    pend = []
                    for kt in range(nkt):
                        j = kt - NKPC * qc
                        col0 = 128 * j if j >= 0 else 0
                        kr = krotc[kt // NKPC]
                        kslice = slice((kt % NKPC) * 128, (kt % NKPC) * 128 + 128)
                        qt = qrot[hp][qc]
                        tslice = slice(col0, QCH)
                        ps = pst.tile([128, 2 * QCH], F32, tag="ps", name="ps")
                        nc.tensor.matmul(
                            ps[:, col0:QCH],
                            kr[0:64, kslice],
                            qt[0:64, tslice],
                            start=True,
                            stop=True,
                            tile_position=(0, 0),
                        )
                        nc.tensor.matmul(
                            ps[:, QCH : 2 * QCH - col0],
                            kr[64:128, kslice],
                            qt[64:128, tslice],
                            start=True,
                            stop=True,
                            tile_position=(64, 0),
                        )
                        et2 = pb.tile([128, 2 * QCH], BF16, tag="et", name="et")
                        nc.scalar.activation(
                            et2[:, col0 : 2 * QCH - col0],
                            ps[:, col0 : 2 * QCH - col0],
                            mybir.ActivationFunctionType.Exp,
                            scale=SCALE,
                        )
                        if j >= 0:
                            nc.vector.tensor_mul(
                                et2[:, col0 : col0 + 128],
                                et2[:, col0 : col0 + 128],
                                triu[:],
                            )
                            nc.vector.tensor_mul(
                                et2[:, QCH : QCH + 128],
                                et2[:, QCH : QCH + 128],
                                triu[:],
                            )
                        pend.append((kt, col0, et2))
                        if len(pend) > 1:
                            emit_avs(pend.pop(0))
                    for e in pend:
                        emit_avs(e)

                    # stash unnormalized output + denominators
                    for i in range(2):
                        h = 2 * hp + i
                        base = 64 * i
                        nc.vector.tensor_copy(
                            outU[hp][qc][base : base + 64, :],
                            po[i][0:HD, :],
                        )
                        nc.vector.tensor_copy(
                            den_all[32 * h : 32 * h + 1, qc * QCH : (qc + 1) * QCH],
                            po[i][HD : HD + 1, :],
                        )

                # per-chunk normalization (exact DVE reciprocal, no ACT tables)
                qs = slice(qc * QCH, (qc + 1) * QCH)
                rec = pb2.tile([97, QCH], F32, tag="rec", name="rec")
                nc.vector.reciprocal(rec[:], den_all[:, qs])
                for h in range(NQ):
                    nc.vector.tensor_copy(invh[h][:, qs], rec[32 * h : 32 * h + 1, :])
                for h in range(NQ):
                    hp, i = h // 2, h % 2
                    base = 64 * i
                    bc = pb2.tile([128, QCH], F32, tag="bc", name="bc")
                    nc.gpsimd.partition_broadcast(bc[:], invh[h][:, qs])
                    nc.vector.tensor_mul(
                        outTn[hp][qc][base : base + 64, :],
                        outU[hp][qc][base : base + 64, :],
                        bc[base : base + 64, :],
                    )

                # per-chunk output projection + store
                for tq in range(qc * NKPC, (qc + 1) * NKPC):
                    ksl = slice((tq % NKPC) * 128, (tq % NKPC) * 128 + 128)
                    for n2 in range(2):
                        py = ppj.tile([128, 512], F32, tag="pq", name="py")
                        nc.tensor.matmul(
                            py[:],
                            outTn[0][qc][:, ksl],
                            wo_s[:, 0, n2 * 512 : (n2 + 1) * 512],
                            start=True,
                            stop=False,
                        )
                        nc.tensor.matmul(
                            py[:],
                            outTn[1][qc][:, ksl],
                            wo_s[:, 1, n2 * 512 : (n2 + 1) * 512],
                            start=False,
                            stop=True,
                        )
                        ys = pb2.tile([128, 512], BF16, tag="ys")
                        nc.vector.tensor_copy(ys[:], py[:])
                        nc.sync.dma_start(
                            out_d[tq * 128 : (tq + 1) * 128, n2 * 512 : (n2 + 1) * 512],
                            ys[:],
                        )
    nc.compile()
    return nc


def _host_tables():
    ev = np.arange(0, HD, 2)
    od = ev + 1
    inv = 1.0 / (THETA ** (np.arange(0, HD, 2, dtype=np.float64) / HD))  # [32]
    freqs = np.outer(inv, np.arange(T, dtype=np.float64))  # [32, T]
    cosq = np.tile(np.cos(freqs), (4, 1)).astype(np.float32)  # [128, T]
    sinq = np.tile(np.sin(freqs), (4, 1)).astype(np.float32)
    triu = np.triu(np.ones((128, 128), np.float32)).astype(ml_dtypes.bfloat16)
    ident = np.eye(128, dtype=np.float32).astype(ml_dtypes.bfloat16)
    return ev, od, cosq, sinq, triu, ident


def make_in_maps(inputs):
    x = np.asarray(inputs["x"], dtype=np.float32)
    wq = np.asarray(inputs["wq"], dtype=np.float32)
    wk = np.asarray(inputs["wk"], dtype=np.float32)
    wv = np.asarray(inputs["wv"], dtype=np.float32)
    wo = np.asarray(inputs["wo"], dtype=np.float32)
    ev, od, cosq, sinq, triu, ident = _host_tables()
    in_maps = []
    for c in range(NCORES):
        b, g = c // 4, c % 4
        qe = np.concatenate([wq[:, 64 * (4 * g + h) + ev] for h in range(NQ)], axis=1)
        qo = np.concatenate([wq[:, 64 * (4 * g + h) + od] for h in range(NQ)], axis=1)
        wqkv_g = np.concatenate(
            [
                qe,
                qo,
                wk[:, 64 * g + ev],
                wk[:, 64 * g + od],
                wv[:, 64 * g : 64 * (g + 1)],
            ],
            axis=1,
        ).astype(ml_dtypes.bfloat16)  # [1024, 384]
        wo_g = wo[256 * g : 256 * (g + 1), :].astype(ml_dtypes.bfloat16)
        in_maps.append(
            {
                "x": np.ascontiguousarray(x[b]).astype(ml_dtypes.bfloat16),
                "wqkv": np.ascontiguousarray(wqkv_g),
                "wo": np.ascontiguousarray(wo_g),
                "cosq": cosq,
                "sinq": sinq,
                "triu": triu,
                "ident": ident,
            }
        )
    return in_maps


_NC_CACHE = None


def kernel(**inputs):
    global _NC_CACHE
    if _NC_CACHE is None:
        _NC_CACHE = build_nc()
    in_maps = make_in_maps(inputs)
    res = run_bass_kernel_spmd(_NC_CACHE, in_maps, list(range(NCORES)))
    out = np.zeros((B, T, DIM), np.float32)
    for c in range(NCORES):
        out[c // 4] += np.asarray(res.results[c]["out"], dtype=np.float32)
    return out


# revision 22
# speedup vs baseline: 1.1004x; 1.1004x over previous
"""GQA attention (RoPE, causal) on 8 TRN2 NeuronCores.

Sharding: core c = (b, g) with b = c // 4 (batch), g = c % 4 (kv-group).
Each core computes 4 query heads + 1 kv head of attention for one batch
element, plus its slice of the output projection; the host sums the 4
partial outputs per batch (row-parallel wo unshard).

Layout tricks:
- wq/wk columns are permuted on the host (per-head de-interleave of RoPE
  pairs). Scores are invariant to a shared per-head permutation of
  head_dim, and this makes RoPE contiguous-block elementwise ops.
- Scores are computed transposed, ST[k, t] = K_rot @ Q_rot^T, so the AV
  matmul consumes exp(ST) directly with V in natural [t, d] layout and a
  ones-column in V yields the softmax denominators for free.
- Matmuls run in bf16 (f32 accumulate); x and the weights are cast to
  bf16 on the host. Softmax skips the max-subtraction (scores are small,
  exp is safe in f32) and normalization is batched per head.
"""
import sys

sys.path.insert(0, "/opt/trn_rl_repo")
import ml_dtypes
import numpy as np

import concourse.bass as bass  # noqa: F401
import concourse.tile as tile
from concourse import bacc, mybir
from concourse.bass_utils import run_bass_kernel_spmd

F32 = mybir.dt.float32
BF16 = mybir.dt.bfloat16

B, T, DIM = 2, 2048, 1024
H, KV, HD = 16, 4, 64
NQ = H // KV          # q heads per core
THETA = 10000.0
SCALE = HD ** -0.5
NCORES = 8
QCH = 512             # q-chunk (free dim of scores/AV matmuls)
NQC = T // QCH        # 4 q-chunks
NKT = T // 128        # 16 k-tiles


def build_nc():
    nc = bacc.Bacc(None, target_bir_lowering=False)
    x_d = nc.declare_dram_parameter("x", [T, DIM], BF16, isOutput=False)
    wqkv_d = nc.declare_dram_parameter("wqkv", [DIM, 384], BF16, isOutput=False)
    wo_d = nc.declare_dram_parameter("wo", [256, DIM], BF16, isOutput=False)
    cosq_d = nc.declare_dram_parameter("cosq", [128, T], F32, isOutput=False)
    sinq_d = nc.declare_dram_parameter("sinq", [128, T], F32, isOutput=False)
    triu_d = nc.declare_dram_parameter("triu", [128, 128], BF16, isOutput=False)
    id_d = nc.declare_dram_parameter("ident", [128, 128], BF16, isOutput=False)
    out_d = nc.declare_dram_parameter("out", [T, DIM], BF16, isOutput=True)

    XCH = 512
    NCH = T // XCH        # 4 chunks
    NKPC = 4              # k-tiles per chunk

    with tile.TileContext(nc) as tc:
        with (
            tc.tile_pool(name="persist", bufs=1) as pp,
            tc.tile_pool(name="vpool", bufs=16) as vp,
            tc.tile_pool(name="xtp", bufs=2) as pxt,
            tc.tile_pool(name="chunk", bufs=2) as pch,
            tc.tile_pool(name="attn", bufs=4) as pb,
            tc.tile_pool(name="small", bufs=2) as pb2,
            tc.tile_pool(name="norm", bufs=1) as pb1,
            tc.tile_pool(name="pst", bufs=2, space="PSUM") as pst,
            tc.tile_pool(name="pav", bufs=2, space="PSUM") as pav,
            tc.tile_pool(name="pproj", bufs=2, space="PSUM") as ppj,
        ):
            # ---- persistent tiles ----
            wqkv_s = pp.tile([128, 8, 384], BF16, tag="wqkv_s")
            nc.sync.dma_start(wqkv_s[:], wqkv_d.rearrange("(k p) c -> p k c", p=128))

            # per-chunk rotated q/k, unnormalized out, normalized out
            qrot = [
                [
                    pp.tile([128, XCH], BF16, tag=f"qr{hp}_{c}", name=f"qr{hp}_{c}")
                    for c in range(NCH)
                ]
                for hp in range(2)
            ]
            krotc = [
                pp.tile([128, XCH], BF16, tag=f"kr{c}", name=f"kr{c}")
                for c in range(NCH)
            ]
            outU = [
                [
                    pp.tile([128, XCH], F32, tag=f"oU{hp}_{c}", name=f"oU{hp}_{c}")
                    for c in range(NCH)
                ]
                for hp in range(2)
            ]
            outTn = [
                [
                    pp.tile([128, XCH], BF16, tag=f"oT{hp}_{c}", name=f"oT{hp}_{c}")
                    for c in range(NCH)
                ]
                for hp in range(2)
            ]
            v_tiles = [
                vp.tile([128, HD + 1], BF16, tag="v", name=f"v{i}")
                for i in range(NKT)
            ]
            ones128 = pp.tile([128, 1], BF16, tag="ones128")
            nc.vector.memset(ones128[:], 1.0)
            ones_row = pp.tile([128, HD], BF16, tag="ones_row")
            nc.vector.memset(ones_row[:], 1.0)
            den_all = pb1.tile([97, T], F32, tag="den_all")
            inv_all = pb1.tile([97, T], BF16, tag="inv_all")

            id_s = pp.tile([128, 128], BF16, tag="ident")
            nc.sync.dma_start(id_s[:], id_d[:])

            # x loaded via regular DMAs; transposed on PE per chunk
            xins = []
            for nch in range(NCH):
                xin = [
                    pxt.tile([128, DIM], BF16, tag="xin", name=f"xin{i}", bufs=6)
                    for i in range(4)
                ]
                for i in range(4):
                    r0 = nch * XCH + i * 128
                    nc.sync.dma_start(xin[i][:], x_d[r0 : r0 + 128, :])
                xins.append(xin)

            cosq = pp.tile([128, T], F32, tag="cosq")
            sinq = pp.tile([128, T], F32, tag="sinq")
            triu = pp.tile([128, 128], BF16, tag="triu")
            nc.sync.dma_start(cosq[:], cosq_d[:])
            nc.sync.dma_start(sinq[:], sinq_d[:])
            nc.sync.dma_start(triu[:], triu_d[:])
            wo_s = pp.tile([128, 2, DIM], BF16, tag="wo_s")
            nc.sync.dma_start(wo_s[:], wo_d.rearrange("(k p) c -> p k c", p=128))

            # ---- phase A: per-chunk projection + rope + v build ----
            for nch in range(NCH):
                cs = slice(nch * XCH, (nch + 1) * XCH)
                ccs = cosq[:, cs]
                scs = sinq[:, cs]
                xtc = [
                    pxt.tile([128, XCH], BF16, tag=f"xt{d}", name=f"xt{d}", bufs=2)
                    for d in range(8)
                ]
                for d in range(8):
                    for i in range(4):
                        ptx = ppj.tile([128, 128], BF16, tag="pq", name="ptx")
                        nc.tensor.transpose(
                            ptx[:],
                            xins[nch][i][:, d * 128 : (d + 1) * 128],
                            id_s[:],
                        )
                        dst = xtc[d][:, i * 128 : (i + 1) * 128]
                        if (d * 4 + i) % 2 == 0:
                            nc.vector.tensor_copy(dst, ptx[:])
                        else:
                            nc.scalar.copy(dst, ptx[:])
                t0c = pch.tile([128, XCH], F32, tag="t0c", name="t0c")
                t1c = pch.tile([128, XCH], F32, tag="t1c", name="t1c")
                tkc = pch.tile([64, XCH], F32, tag="tkc", name="tkc")
                vTc = pch.tile([64, XCH], BF16, tag="vTc", name="vTc")
                for m in range(3):
                    pq = ppj.tile([128, XCH], F32, tag="pq", name="pq")
                    for k in range(8):
                        nc.tensor.matmul(
                            pq[:],
                            wqkv_s[:, k, m * 128 : (m + 1) * 128],
                            xtc[k][:],
                            start=(k == 0),
                            stop=(k == 7),
                        )
                    if m == 0:
                        nc.scalar.copy(t0c[:], pq[:])
                    elif m == 1:
                        nc.scalar.copy(t1c[:], pq[:])
                    else:
                        nc.vector.tensor_copy(tkc[0:64, :], pq[0:64, :])
                        nc.vector.tensor_copy(vTc[:, :], pq[64:128, :])

                # RoPE (q) for this chunk
                sA = pch.tile([128, XCH], F32, tag="sA", name="sA")
                sB = pch.tile([128, XCH], F32, tag="sB", name="sB")
                nc.vector.tensor_mul(sA[:], t0c[:], ccs)
                nc.vector.tensor_mul(sB[:], t1c[:], scs)
                nc.vector.tensor_sub(sA[:], sA[:], sB[:])      # rotated evens
                nc.vector.tensor_mul(sB[:], t0c[:], scs)
                nc.vector.tensor_mul(t0c[:], t1c[:], ccs)
                nc.vector.tensor_add(sB[:], sB[:], t0c[:])     # rotated odds
                for h in range(NQ):
                    qt = qrot[h // 2][nch]
                    roff = 64 * (h % 2)
                    src_e = sA[32 * h : 32 * h + 32, :]
                    src_o = sB[32 * h : 32 * h + 32, :]
                    if h % 2 == 0:
                        nc.vector.tensor_copy(qt[roff : roff + 32, :], src_e)
                        nc.scalar.copy(qt[roff + 32 : roff + 64, :], src_o)
                    else:
                        nc.scalar.copy(qt[roff : roff + 32, :], src_e)
                        nc.vector.tensor_copy(qt[roff + 32 : roff + 64, :], src_o)

                # RoPE (k) for this chunk, duplicated rows 64:128
                krc = krotc[nch]
                nc.vector.tensor_mul(sA[0:32, :], tkc[0:32, :], ccs[0:32, :])
                nc.vector.tensor_mul(sB[0:32, :], tkc[32:64, :], scs[32:64, :])
                nc.vector.tensor_sub(krc[0:32, :], sA[0:32, :], sB[0:32, :])
                nc.vector.tensor_mul(sA[0:32, :], tkc[0:32, :], scs[0:32, :])
                nc.vector.tensor_mul(sB[0:32, :], tkc[32:64, :], ccs[32:64, :])
                nc.vector.tensor_add(krc[32:64, :], sA[0:32, :], sB[0:32, :])
                nc.vector.tensor_copy(krc[64:128, :], krc[0:64, :])

                # V tiles for this chunk
                for ii in range(NKPC):
                    i = nch * NKPC + ii
                    pv = ppj.tile([128, HD], BF16, tag="pq", name="pv")
                    nc.tensor.transpose(
                        pv[:],
                        vTc[:, ii * 128 : (ii + 1) * 128],
                        id_s[0:64, 0:64],
                    )
                    nc.vector.tensor_copy(v_tiles[i][:, 0:HD], pv[:])
                    nc.scalar.copy(v_tiles[i][:, HD : HD + 1], ones128[:])

            # ---- phase B: attention (qc-outer, 2-stage skew, paired heads) ----
            for qc in range(NQC):
                nkt = NKPC * (qc + 1)
                for hp in range(2):
                    po = [
                        pav.tile([HD + 1, QCH], F32, tag="po", name=f"po{i}")
                        for i in range(2)
                    ]

                    def emit_avs(entry):
                        ktv, col0, et2 = entry
                        nc.tensor.matmul(
                            po[0][:, col0:QCH],
                            v_tiles[ktv][:],
                            et2[:, col0:QCH],
                            start=(ktv == 0),
                            stop=(ktv == nkt - 1),
                        )
                        nc.tensor.matmul(
                            po[1][:, col0:QCH],
                            v_tiles[ktv][:],
                            et2[:, QCH : 2 * QCH - col0],
                            start=(ktv == 0),
                            stop=(ktv == nkt - 1),
                        )

                    pend = []
                    for kt in range(nkt):
                        j = kt - NKPC * qc
                        col0 = 128 * j if j >= 0 else 0
                        kr = krotc[kt // NKPC]
                        kslice = slice((kt % NKPC) * 128, (kt % NKPC) * 128 + 128)
                        qt = qrot[hp][qc]
                        tslice = slice(col0, QCH)
                        ps = pst.tile([128, 2 * QCH], F32, tag="ps", name="ps")
                        nc.tensor.matmul(
                            ps[:, col0:QCH],
                            kr[0:64, kslice],
                            qt[0:64, tslice],
                            start=True,
                            stop=True,
                            tile_position=(0, 0),
                        )
                        nc.tensor.matmul(
                            ps[:, QCH : 2 * QCH - col0],
                            kr[64:128, kslice],
                            qt[64:128, tslice],
                            start=True,
                            stop=True,
                            tile_position=(64, 0),
                        )
                        et2 = pb.tile([128, 2 * QCH], BF16, tag="et", name="et")
                        nc.scalar.activation(
                            et2[:, col0 : 2 * QCH - col0],
                            ps[:, col0 : 2 * QCH - col0],
                            mybir.ActivationFunctionType.Exp,
                            scale=SCALE,
                        )
                        if j >= 0:
                            nc.vector.tensor_mul(
                                et2[:, col0 : col0 + 128],
                                et2[:, col0 : col0 + 128],
                                triu[:],
                            )
                            nc.vector.tensor_mul(
                                et2[:, QCH : QCH + 128],
                                et2[:, QCH : QCH + 128],
                                triu[:],
                            )
                        pend.append((kt, col0, et2))
                        if len(pend) > 1:
                            emit_avs(pend.pop(0))
                    for e in pend:
                        emit_avs(e)

                    # stash unnormalized output + denominators
                    for i in range(2):
                        h = 2 * hp + i
                        base = 64 * i
                        nc.vector.tensor_copy(
                            outU[hp][qc][base : base + 64, :],
                            po[i][0:HD, :],
                        )
                        nc.vector.tensor_copy(
                            den_all[32 * h : 32 * h + 1, qc * QCH : (qc + 1) * QCH],
                            po[i][HD : HD + 1, :],
                        )

                # per-chunk normalization (exact DVE reciprocal, no ACT tables)
                qs = slice(qc * QCH, (qc + 1) * QCH)
                with nc.allow_low_precision(reason="softmax denominators fit bf16"):
                    nc.vector.reciprocal(inv_all[:, qs], den_all[:, qs])
                for h in range(NQ):
                    hp, i = h // 2, h % 2
                    base = 64 * i
                    r = 32 * h
                    bc = ppj.tile([HD, QCH], F32, tag="pq", name="bc")
                    nc.tensor.matmul(
                        bc[:],
                        ones_row[r : r + 1, :],
                        inv_all[r : r + 1, qs],
                        start=True,
                        stop=True,
                        tile_position=(r, 0),
                    )
                    nc.vector.tensor_mul(
                        outTn[hp][qc][base : base + 64, :],
                        outU[hp][qc][base : base + 64, :],
                        bc[:],
                    )

                # per-chunk output projection + store
                for tq in range(qc * NKPC, (qc + 1) * NKPC):
                    ksl = slice((tq % NKPC) * 128, (tq % NKPC) * 128 + 128)
                    for n2 in range(2):
                        py = ppj.tile([128, 512], F32, tag="pq", name="py")
                        nc.tensor.matmul(
                            py[:],
                            outTn[0][qc][:, ksl],
                            wo_s[:, 0, n2 * 512 : (n2 + 1) * 512],
                            start=True,
                            stop=False,
                        )
                        nc.tensor.matmul(
                            py[:],
                            outTn[1][qc][:, ksl],
                            wo_s[:, 1, n2 * 512 : (n2 + 1) * 512],
                            start=False,
                            stop=True,
                        )
                        ys = pb2.tile([128, 512], BF16, tag="ys")
                        nc.vector.tensor_copy(ys[:], py[:])
                        nc.sync.dma_start(
                            out_d[tq * 128 : (tq + 1) * 128, n2 * 512 : (n2 + 1) * 512],
                            ys[:],
                        )
    nc.compile()
    return nc


def _host_tables():
    ev = np.arange(0, HD, 2)
    od = ev + 1
    inv = 1.0 / (THETA ** (np.arange(0, HD, 2, dtype=np.float64) / HD))  # [32]
    freqs = np.outer(inv, np.arange(T, dtype=np.float64))  # [32, T]
    cosq = np.tile(np.cos(freqs), (4, 1)).astype(np.float32)  # [128, T]
    sinq = np.tile(np.sin(freqs), (4, 1)).astype(np.float32)
    triu = np.triu(np.ones((128, 128), np.float32)).astype(ml_dtypes.bfloat16)
    ident = np.eye(128, dtype=np.float32).astype(ml_dtypes.bfloat16)
    return ev, od, cosq, sinq, triu, ident


def make_in_maps(inputs):
    x = np.asarray(inputs["x"], dtype=np.float32)
    wq = np.asarray(inputs["wq"], dtype=np.float32)
    wk = np.asarray(inputs["wk"], dtype=np.float32)
    wv = np.asarray(inputs["wv"], dtype=np.float32)
    wo = np.asarray(inputs["wo"], dtype=np.float32)
    ev, od, cosq, sinq, triu, ident = _host_tables()
    in_maps = []
    for c in range(NCORES):
        b, g = c // 4, c % 4
        qe = np.concatenate([wq[:, 64 * (4 * g + h) + ev] for h in range(NQ)], axis=1)
        qo = np.concatenate([wq[:, 64 * (4 * g + h) + od] for h in range(NQ)], axis=1)
        wqkv_g = np.concatenate(
            [
                qe,
                qo,
                wk[:, 64 * g + ev],
                wk[:, 64 * g + od],
                wv[:, 64 * g : 64 * (g + 1)],
            ],
            axis=1,
        ).astype(ml_dtypes.bfloat16)  # [1024, 384]
        wo_g = wo[256 * g : 256 * (g + 1), :].astype(ml_dtypes.bfloat16)
        in_maps.append(
            {
                "x": np.ascontiguousarray(x[b]).astype(ml_dtypes.bfloat16),
                "wqkv": np.ascontiguousarray(wqkv_g),
                "wo": np.ascontiguousarray(wo_g),
                "cosq": cosq,
                "sinq": sinq,
                "triu": triu,
                "ident": ident,
            }
        )
    return in_maps


_NC_CACHE = None


def kernel(**inputs):
    global _NC_CACHE
    if _NC_CACHE is None:
        _NC_CACHE = build_nc()
    in_maps = make_in_maps(inputs)
    res = run_bass_kernel_spmd(_NC_CACHE, in_maps, list(range(NCORES)))
    out = np.zeros((B, T, DIM), np.float32)
    for c in range(NCORES):
        out[c // 4] += np.asarray(res.results[c]["out"], dtype=np.float32)
    return out
